# revision 18
# baseline (speedup 1.0000x reference)
"""Trainium2 Bass kernel for nn_SoftBiasTransformer.

3-layer post-norm transformer encoder, B=1024 S=64 D=768 H=6 HD=128 FF=3072,
with a learned [S,S] additive attention bias shared across batch/heads.

Strategy:
- Data-parallel over batch across 8 NeuronCores (128 batches/core),
  processed in 8 chunks of 1024 tokens; full weights re-streamed per chunk
  (overlapped with compute).
- fp8-e4m3 DoubleRow matmuls (2x TensorE stream rate) for the Q/K/V/Wo
  projections and the LayerNorm statistics; fp16 for the FFN (fp8 there
  breaches the 2e-2 gate: measured 2.5e-2 in simulation) and for the small
  attention matmuls. fp32 PSUM accumulation everywhere.
- Weights pre-scaled by 64 into fp8 range (descale folded into the PSUM
  evacuation); attention context pre-scaled by 16 into fp8 (rms ~0.07),
  descaled in the Wo evacuation.
- Feature-major activations [D(part), tokens(free)]: dense chain computes
  outT = W.T @ xT with weights stationary, no activation transposes. V is
  produced token-major (x stationary) so the attention context matmul can
  consume PE-transposed softmax probabilities directly.
- LayerNorm mean/mean-square via fp8 ones-vector DoubleRow matmuls
  (reduction over the feature/partition axis), broadcast back with K=1
  matmuls. LN1 statistic casts/matmuls are interleaved into the Wo loop
  (q-outer) and LN2's into the FFN q-loop, so the serial stats->var->rstd
  chain hides behind the other half's matmuls instead of stalling the PE.
- Softmax bias folded in multiplicatively: exp(s+b) = exp(s)*exp(b), with
  exp(bias) precomputed on the host. The 1/sqrt(HD) score scale is folded
  into Wq/bq on the host.
- Attention emission is pipelined (scores run 2 heads ahead, V-projection
  second half interleaved) so the softmax Scalar/Vector chain hides behind
  TensorE work. Output is stored via PE transposes in fp16.
"""

import math

import numpy as np

B, S, D = 1024, 64, 768
H, HD, FF, L = 6, 128, 3072, 3
NCORES = 8
BS = B // NCORES            # batches per core = 128
TOK = BS * S                # tokens per core = 8192
KD = D // 128               # 6
KF = FF // 128              # 24
EPS = 1e-5
TC = 1024                   # tokens per chunk
BC = TC // S                # batches per chunk = 16
NQ = TC // 512              # 512-col units per chunk = 2
T8 = TC // 128              # 128-token blocks per chunk = 8
W8 = 64.0                   # fp8 weight pre-scale
CTX8 = 16.0                 # fp8 context pre-scale

_CACHED_NC = {}


def _build_nc(n_chunks, stage="full"):
    import concourse.tile as tile
    import concourse.mybir as mybir
    from concourse import bacc
    from contextlib import ExitStack

    f8 = mybir.dt.float8e4
    f16 = mybir.dt.float16
    f32 = mybir.dt.float32
    Alu = mybir.AluOpType
    Act = mybir.ActivationFunctionType
    DR = mybir.MatmulPerfMode.DoubleRow

    nc = bacc.Bacc("TRN2", target_bir_lowering=False, debug=False,
                   enable_asserts=False, num_devices=1)

    # ---- DRAM I/O ----
    xw = nc.dram_tensor("xw", [KD, 128, TOK], f16, kind="ExternalInput")
    xw8 = nc.dram_tensor("xw8", [KD, 128, TOK], f8, kind="ExternalInput")
    Wq8 = nc.dram_tensor("Wq8", [L, KD, 128, D], f8, kind="ExternalInput")
    Wk8 = nc.dram_tensor("Wk8", [L, KD, 128, D], f8, kind="ExternalInput")
    Wv8 = nc.dram_tensor("Wv8", [L, KD, 128, D], f8, kind="ExternalInput")
    Wo8 = nc.dram_tensor("Wo8", [L, KD, 128, D], f8, kind="ExternalInput")
    W1_s = nc.dram_tensor("W1_s", [L, KD, 128, FF], f16, kind="ExternalInput")
    W2_s = nc.dram_tensor("W2_s", [L, KF, 128, D], f16, kind="ExternalInput")
    bq_t = nc.dram_tensor("bq_t", [L, 128, KD], f32, kind="ExternalInput")
    bk_t = nc.dram_tensor("bk_t", [L, 128, KD], f32, kind="ExternalInput")
    bv16_t = nc.dram_tensor("bv16_t", [L, 128, KD], f32, kind="ExternalInput")
    bo_t = nc.dram_tensor("bo_t", [L, 128, KD], f32, kind="ExternalInput")
    b1_t = nc.dram_tensor("b1_t", [L, 128, KF], f32, kind="ExternalInput")
    b2_t = nc.dram_tensor("b2_t", [L, 128, KD], f32, kind="ExternalInput")
    l1s_t = nc.dram_tensor("l1s_t", [L, 128, KD], f32, kind="ExternalInput")
    l1b_t = nc.dram_tensor("l1b_t", [L, 128, KD], f32, kind="ExternalInput")
    l2s_t = nc.dram_tensor("l2s_t", [L, 128, KD], f32, kind="ExternalInput")
    l2b_t = nc.dram_tensor("l2b_t", [L, 128, KD], f32, kind="ExternalInput")
    eb2 = nc.dram_tensor("eb2", [128, S], f16, kind="ExternalInput")
    id16 = nc.dram_tensor("id16", [128, 128], f16, kind="ExternalInput")
    y = nc.dram_tensor("y", [TOK, D], f16, kind="ExternalOutput")

    inv_d = 1.0 / D

    with tile.TileContext(nc) as tc, ExitStack() as ctx:
        consts = ctx.enter_context(tc.tile_pool(name="consts", bufs=1))
        p_x = ctx.enter_context(tc.tile_pool(name="p_x", bufs=1))
        p_act = ctx.enter_context(tc.tile_pool(name="p_act", bufs=1))
        p_sm = ctx.enter_context(tc.tile_pool(name="p_sm", bufs=1))
        p_sq = ctx.enter_context(tc.tile_pool(name="p_sq", bufs=2))
        p_h = ctx.enter_context(tc.tile_pool(name="p_h", bufs=1))
        p_row = ctx.enter_context(tc.tile_pool(name="p_row", bufs=2))
        p_out = ctx.enter_context(tc.tile_pool(name="p_out", bufs=2))
        p_w = ctx.enter_context(tc.tile_pool(name="p_w", bufs=2))
        p_w1 = ctx.enter_context(tc.tile_pool(name="p_w1", bufs=1))
        p_w2 = ctx.enter_context(tc.tile_pool(name="p_w2", bufs=1))
        p_s8 = ctx.enter_context(tc.tile_pool(name="p_s8", bufs=1))
        ps_p = ctx.enter_context(tc.tile_pool(name="ps_p", bufs=1, space="PSUM"))

        def ps_tile(shape, dtype, tag, bufs):
            t = ps_p.tile(shape, dtype, tag=tag, bufs=bufs, name=f"ps_{tag}")
            return t

        # ---- constants ----
        ones8 = consts.tile([128, 2, 16], f8)      # 1.0: LN stats DR lhsT
        nc.vector.memset(ones8, 1.0)
        ones_row = consts.tile([1, 128], f16)      # K=1 broadcast lhsT
        nc.vector.memset(ones_row, 1.0)
        eps_t = consts.tile([1, 1], f32)
        nc.vector.memset(eps_t, EPS)
        eb2_sb = consts.tile([128, S], f16)
        nc.sync.dma_start(eb2_sb[:], eb2.ap())
        id16_sb = consts.tile([128, 128], f16)
        nc.sync.dma_start(id16_sb[:], id16.ap())

        per_layer = {}
        for l in range(L):
            d = {}
            for name, dram, w in [
                ("bq", bq_t, KD), ("bk", bk_t, KD), ("bv16", bv16_t, KD),
                ("bo", bo_t, KD), ("b1", b1_t, KF), ("b2", b2_t, KD),
                ("l1s", l1s_t, KD), ("l1b", l1b_t, KD),
                ("l2s", l2s_t, KD), ("l2b", l2b_t, KD),
            ]:
                t = consts.tile([128, w], f32, tag=f"{name}_{l}")
                nc.sync.dma_start(t[:], dram.ap()[l])
                d[name] = t
            per_layer[l] = d

        def load_w8(dram, l, tag="wqkvo"):
            t = p_w.tile([128, KD, D], f8, tag=tag, name="w8sb")
            nc.sync.dma_start(t[:], dram.ap()[l].rearrange("k p f -> p k f"))
            return t

        def ln_alloc(want_out8):
            s8 = p_x.tile([128, KD, TC], f8, tag="x8", name="s8")
            sq8 = p_s8.tile([128, KD, TC], f8, tag="sq8", name="sq8")
            out8 = (p_x.tile([128, KD, TC], f8, tag="x8", name="out8")
                    if want_out8 else None)
            return s8, sq8, out8

        def ln_cast(s8, sq8, s_in, m, sl):
            nc.vector.tensor_copy(s8[:, m, sl], s_in[:, m, sl])
            nc.vector.tensor_tensor(sq8[:, m, sl], s8[:, m, sl],
                                    s8[:, m, sl], Alu.mult)

        def ln_stats(s8, sq8, half):
            sl = slice(half * 512, half * 512 + 512)
            mu_ps = ps_tile([1, 512], f32, "st", 2)
            for kp in range(KD // 2):
                nc.tensor.matmul(mu_ps[:], ones8[:, :, 0:1],
                                 s8[:, 2 * kp:2 * kp + 2, sl],
                                 start=(kp == 0), stop=(kp == KD // 2 - 1),
                                 perf_mode=DR)
            msq_ps = ps_tile([1, 512], f32, "st", 2)
            for kp in range(KD // 2):
                nc.tensor.matmul(msq_ps[:], ones8[:, :, 0:1],
                                 sq8[:, 2 * kp:2 * kp + 2, sl],
                                 start=(kp == 0), stop=(kp == KD // 2 - 1),
                                 perf_mode=DR)
            musq = p_row.tile([1, 512], f32, tag="row32")
            nc.scalar.activation(musq[:], mu_ps[:], Act.Square, scale=inv_d)
            mu16 = p_row.tile([1, 512], f16, tag="mu16")
            nc.scalar.activation(mu16[:], mu_ps[:], Act.Copy, scale=inv_d)
            var = p_row.tile([1, 512], f32, tag="row32")
            nc.vector.scalar_tensor_tensor(var[:], msq_ps[:], inv_d,
                                           musq[:], Alu.mult, Alu.subtract)
            std = p_row.tile([1, 512], f32, tag="row32")
            nc.scalar.activation(std[:], var[:], Act.Sqrt, bias=eps_t[:])
            rstd = p_row.tile([1, 512], f16, tag="rstd")
            with nc.allow_low_precision(reason="fp16 rstd for broadcast"):
                nc.vector.reciprocal(rstd[:], std[:])
            return mu16, rstd

        def ln_bc(ch):
            mu16, rstd = ch
            mu_b = ps_tile([128, 512], f32, "mm", 4)
            nc.tensor.matmul(mu_b[:], ones_row[:], mu16[:],
                             start=True, stop=True)
            rstd_b = ps_tile([128, 512], f32, "mm", 4)
            nc.tensor.matmul(rstd_b[:], ones_row[:], rstd[:],
                             start=True, stop=True)
            return mu_b, rstd_b

        def ln_norm(s_in, gamma, beta, out_sb, out8, half, bc):
            sl = slice(half * 512, half * 512 + 512)
            mu_b, rstd_b = bc
            for m in range(KD):
                t0 = p_sq.tile([128, 512], f16, tag="lnt")
                nc.vector.scalar_tensor_tensor(
                    t0[:], s_in[:, m, sl], 1.0, mu_b[:],
                    Alu.mult, Alu.subtract)
                nc.vector.scalar_tensor_tensor(
                    t0[:], t0[:], gamma[:, m:m + 1], rstd_b[:],
                    Alu.mult, Alu.mult)
                nc.vector.tensor_scalar_add(
                    out_sb[:, m, sl], t0[:], beta[:, m:m + 1])
                if out8 is not None:
                    nc.vector.tensor_scalar_add(
                        out8[:, m, sl], t0[:], beta[:, m:m + 1])

        def dump(tile_f16, c):
            """Debug: cast a [128,*] tile to f32 and DMA into y (flat)."""
            yf = y.ap().rearrange("t d -> (t d)").rearrange(
                "(p f) -> p f", p=128)
            flat = tile_f16[:]
            if len(flat.shape) == 3:
                flat = flat.rearrange("p a b -> p (a b)")
            np_, n = flat.shape
            for q in range(n // 512):
                t32 = p_out.tile([128, 512], f16, tag="dump")
                nc.scalar.activation(t32[:np_], flat[:, q * 512:(q + 1) * 512],
                                     Act.Copy)
                nc.sync.dma_start(
                    yf[:np_, c * n + q * 512: c * n + (q + 1) * 512],
                    t32[:np_])

        # ---------------- main program ----------------
        for c in range(n_chunks):
            tok0 = c * TC
            x16 = p_x.tile([128, KD, TC], f16, tag="x16")
            nc.sync.dma_start(
                x16[:], xw.ap()[:, :, tok0:tok0 + TC].rearrange("o p t -> p o t"))
            x8 = p_x.tile([128, KD, TC], f8, tag="x8")
            nc.sync.dma_start(
                x8[:], xw8.ap()[:, :, tok0:tok0 + TC].rearrange("o p t -> p o t"))

            for l in range(L):
                cl = per_layer[l]
                xin = x16

                # --- Q, K projections (feature-major, fp8 DoubleRow) ---
                wq = load_w8(Wq8, l)
                q16 = p_act.tile([128, KD, TC], f16, tag="q16")
                wk = load_w8(Wk8, l)
                k16 = p_act.tile([128, KD, TC], f16, tag="k16")
                for w_sl, out_sb, bias in ((wq, q16, cl["bq"]),
                                           (wk, k16, cl["bk"])):
                    for m in range(KD):
                        for q in range(NQ):
                            ps = ps_tile([128, 512], f32, "mm", 4)
                            for kp in range(KD // 2):
                                nc.tensor.matmul(
                                    ps[:],
                                    w_sl[:, 2 * kp:2 * kp + 2,
                                         m * 128:(m + 1) * 128],
                                    x8[:, 2 * kp:2 * kp + 2,
                                       q * 512:(q + 1) * 512],
                                    start=(kp == 0), stop=(kp == KD // 2 - 1),
                                    perf_mode=DR)
                            nc.scalar.activation(
                                out_sb[:, m, q * 512:(q + 1) * 512], ps[:],
                                Act.Identity, bias=bias[:, m:m + 1],
                                scale=1.0 / W8)

                if stage == "qk":
                    dump(q16, c)
                    break

                # --- V token-major per batch (fp8 DR): v[s, batch, hd] ---
                wv = load_w8(Wv8, l)
                wo = load_w8(Wo8, l)
                v16a = p_act.tile([64, BC // 2, D], f16, tag="v16")
                v16b = p_h.tile([64, BC // 2, D], f16, tag="h16")
                ctx8 = p_act.tile([128, KD, TC], f8, tag="ctx")

                def vslice(b, cols):
                    t = v16a if b < BC // 2 else v16b
                    return t[:, b % (BC // 2), cols]

                def emit_v(nh):
                    for b in range(BC):
                        ps = ps_tile([64, 384], f32, "mm", 4)
                        for kp in range(KD // 2):
                            nc.tensor.matmul(
                                ps[:],
                                x8[:, 2 * kp:2 * kp + 2, b * 64:(b + 1) * 64],
                                wv[:, 2 * kp:2 * kp + 2,
                                   nh * 384:(nh + 1) * 384],
                                start=(kp == 0), stop=(kp == KD // 2 - 1),
                                perf_mode=DR)
                        nc.vector.tensor_scalar_mul(
                            vslice(b, slice(nh * 384, (nh + 1) * 384)),
                            ps[:], 1.0 / W8)

                def emit_scores(h):
                    """scores + softmax numerators for head h -> ex tile."""
                    sc_ps = ps_tile([128, 512], f32, "sc", 2)
                    for b in range(BC):
                        p_slot = b % 2
                        j = b // 2
                        nc.tensor.matmul(
                            sc_ps[64 * p_slot:64 * p_slot + 64,
                                  j * 64:(j + 1) * 64],
                            q16[:, h, b * 64:(b + 1) * 64],
                            k16[:, h, b * 64:(b + 1) * 64],
                            start=True, stop=True,
                            tile_position=(0, 64 * p_slot))
                    # softmax over keys: p = exp(s)*exp(bias) / sum
                    ex = p_sm.tile([128, 8, S], f16, tag="ex", bufs=3)
                    nc.scalar.activation(
                        ex[:].rearrange("p a b -> p (a b)"), sc_ps[:], Act.Exp)
                    nc.vector.tensor_tensor(
                        ex[:], ex[:],
                        eb2_sb[:, None, :].to_broadcast((128, 8, S)),
                        Alu.mult)
                    sums = p_row.tile([128, 8], f32, tag="sums")
                    nc.vector.reduce_sum(sums[:], ex[:],
                                         axis=mybir.AxisListType.X)
                    rec = p_row.tile([128, 8], f32, tag="rec")
                    nc.vector.reciprocal(rec[:], sums[:])
                    for j in range(8):
                        nc.vector.tensor_scalar_mul(
                            ex[:, j, :], ex[:, j, :], rec[:, j:j + 1])
                    return ex

                def emit_tail(h, ex):
                    """prob transpose + context matmuls + fp8 ctx write."""
                    pTs = []
                    for halfj in range(2):
                        tp_ps = ps_tile([128, 512], f16, "mm", 4)
                        for jj in range(4):
                            j = halfj * 4 + jj
                            nc.tensor.transpose(
                                tp_ps[:64, jj * 128:(jj + 1) * 128],
                                ex[:, j, :], id16_sb[:])
                        pT = p_sm.tile([64, 512], f16, tag="pT", bufs=4)
                        nc.vector.tensor_copy(pT[:], tp_ps[:64, :])
                        pTs.append(pT)
                    for half in range(NQ):
                        cx_ps = ps_tile([128, 512], f32, "mm", 4)
                        for bb in range(8):
                            b = half * 8 + bb
                            p_slot = b % 2
                            j = b // 2
                            pT = pTs[j // 4]
                            nc.tensor.matmul(
                                cx_ps[:, bb * 64:(bb + 1) * 64],
                                vslice(b, slice(h * 128, (h + 1) * 128)),
                                pT[:, (j % 4) * 128 + 64 * p_slot:
                                   (j % 4) * 128 + 64 * p_slot + 64],
                                start=True, stop=True)
                        nc.scalar.activation(
                            ctx8[:, h, half * 512:(half + 1) * 512],
                            cx_ps[:], Act.Identity,
                            bias=cl["bv16"][:, h:h + 1], scale=CTX8)

                # pipelined attention emission
                emit_v(0)
                exs = {0: emit_scores(0), 1: emit_scores(1)}
                emit_v(1)
                if stage == "v":
                    dump(v16a, c)
                    break
                brk = False
                for h in range(H):
                    if h + 2 < H:
                        exs[h + 2] = emit_scores(h + 2)
                    if stage == "attn_sm" and h == 0:
                        dump(exs[0], c)
                        brk = True
                        break
                    emit_tail(h, exs.pop(h))
                if brk:
                    break
                if stage == "attn":
                    dump(ctx8, c)
                    break

                # --- Wo (fp8 DR on ctx8) + residual -> s1, then LN1
                #     (q-outer so LN1 stats of half 0 hide behind Wo half 1)
                s1 = p_act.tile([128, KD, TC], f16, tag="q16")
                s8_1, sq8_1, _ = ln_alloc(False)
                for q in range(NQ):
                    qsl = slice(q * 512, q * 512 + 512)
                    for m in range(KD):
                        ps = ps_tile([128, 512], f32, "mm", 4)
                        for kp in range(KD // 2):
                            nc.tensor.matmul(
                                ps[:],
                                wo[:, 2 * kp:2 * kp + 2,
                                   m * 128:(m + 1) * 128],
                                ctx8[:, 2 * kp:2 * kp + 2, qsl],
                                start=(kp == 0), stop=(kp == KD // 2 - 1),
                                perf_mode=DR)
                        t0 = p_sq.tile([128, 512], f16, tag="lnt")
                        nc.scalar.activation(
                            t0[:], ps[:], Act.Identity,
                            bias=cl["bo"][:, m:m + 1],
                            scale=1.0 / (W8 * CTX8))
                        nc.vector.tensor_tensor(
                            s1[:, m, qsl], t0[:],
                            xin[:, m, qsl], Alu.add)
                        ln_cast(s8_1, sq8_1, s1, m, qsl)
                z16 = p_act.tile([128, KD, TC], f16, tag="k16")
                chs1 = [ln_stats(s8_1, sq8_1, 0), ln_stats(s8_1, sq8_1, 1)]
                bc0 = ln_bc(chs1[0])
                ln_norm(s1, cl["l1s"], cl["l1b"], z16, None, 0, bc0)
                bc1 = ln_bc(chs1[1])
                ln_norm(s1, cl["l1s"], cl["l1b"], z16, None, 1, bc1)
                if stage == "ln1":
                    dump(z16, c)
                    break

                # --- FFN (fp16) ---
                w1 = p_w1.tile([128, KD, FF], f16, tag="w1")
                nc.sync.dma_start(w1[:],
                                  W1_s.ap()[l].rearrange("k p f -> p k f"))
                w2 = p_w2.tile([128, KF, D], f16, tag="w2")
                nc.sync.dma_start(w2[:],
                                  W2_s.ap()[l].rearrange("k p f -> p k f"))
                last = (l == L - 1)
                if not last:
                    xout = p_x.tile([128, KD, TC], f16, tag="x16")
                s2 = p_act.tile([128, KD, TC], f16, tag="v16")
                s8_2, sq8_2, xout8 = ln_alloc(not last)
                chs2 = []
                for q in range(NQ):
                    qsl = slice(q * 512, q * 512 + 512)
                    h16 = p_h.tile([128, KF, 512], f16, tag="h16")
                    for m in range(KF):
                        ps = ps_tile([128, 512], f32, "mm", 4)
                        for k in range(KD):
                            nc.tensor.matmul(
                                ps[:],
                                w1[:, k, m * 128:(m + 1) * 128],
                                z16[:, k, qsl],
                                start=(k == 0), stop=(k == KD - 1))
                        nc.scalar.activation(
                            h16[:, m, :], ps[:], Act.Relu,
                            bias=cl["b1"][:, m:m + 1], scale=1.0)
                    for m in range(KD):
                        ps = ps_tile([128, 512], f32, "mm", 4)
                        for k in range(KF):
                            nc.tensor.matmul(
                                ps[:],
                                w2[:, k, m * 128:(m + 1) * 128],
                                h16[:, k, :],
                                start=(k == 0), stop=(k == KF - 1))
                        nc.vector.scalar_tensor_tensor(
                            s2[:, m, qsl], ps[:], cl["b2"][:, m:m + 1],
                            z16[:, m, qsl], Alu.add, Alu.add)
                        ln_cast(s8_2, sq8_2, s2, m, qsl)
                    # LN2 stats of this half hide behind the other half FFN
                    chs2.append(ln_stats(s8_2, sq8_2, q))

                if stage == "ffn":
                    dump(s2, c)
                    break

                # --- LN2 normalize + output ---
                if last:
                    x2 = p_x.tile([128, KD, TC], f16, tag="x16", name="x2")
                    bc0 = ln_bc(chs2[0])
                    ln_norm(s2, cl["l2s"], cl["l2b"], x2, None, 0, bc0)
                    bc1 = ln_bc(chs2[1])
                    ln_norm(s2, cl["l2s"], cl["l2b"], x2, None, 1, bc1)
                    # transpose to token-major f16 and store
                    for t in range(T8):
                        ps_a = ps_tile([128, 512], f16, "mm", 4)
                        ps_b = ps_tile([128, 512], f16, "mm", 4)
                        for po in range(KD):
                            tgt = ps_a if po < 4 else ps_b
                            off = (po % 4) * 128
                            nc.tensor.transpose(
                                tgt[:, off:off + 128],
                                x2[:, po, t * 128:(t + 1) * 128],
                                id16_sb[:])
                        ob = p_out.tile([128, KD, 128], f16, tag="ob")
                        nc.scalar.activation(
                            ob[:, :4, :].rearrange("p a b -> p (a b)"),
                            ps_a[:], Act.Copy)
                        nc.scalar.activation(
                            ob[:, 4:, :].rearrange("p a b -> p (a b)"),
                            ps_b[:, :256], Act.Copy)
                        nc.sync.dma_start(
                            y.ap()[tok0 + t * 128: tok0 + (t + 1) * 128, :],
                            ob[:].rearrange("p a b -> p (a b)"))
                else:
                    bc0 = ln_bc(chs2[0])
                    ln_norm(s2, cl["l2s"], cl["l2b"], xout, xout8, 0, bc0)
                    bc1 = ln_bc(chs2[1])
                    ln_norm(s2, cl["l2s"], cl["l2b"], xout, xout8, 1, bc1)
                    x16 = xout
                    x8 = xout8

    nc.finalize()
    return nc


def _host_prep(inputs):
    x = np.asarray(inputs["x"])
    scale = 1.0 / math.sqrt(HD)
    f16 = np.float16
    f32 = np.float32
    import ml_dtypes
    f8 = ml_dtypes.float8_e4m3

    def slabs8(w, extra=1.0):
        a = np.asarray(w, f32).reshape(L, KD, 128, np.asarray(w).shape[-1])
        a = np.clip(a * (W8 * extra), -240.0, 240.0)
        return np.ascontiguousarray(a).astype(f8)

    def slabs16(w, nk):
        return np.ascontiguousarray(
            np.asarray(w).reshape(L, nk, 128, np.asarray(w).shape[-1])
        ).astype(f16)

    def cols(b, nk):  # [L, feat] -> [L, 128, nk]
        return np.ascontiguousarray(
            np.asarray(b, f32).reshape(L, nk, 128).transpose(0, 2, 1))

    prep = {
        "Wq8": slabs8(inputs["Wq"], extra=scale),
        "Wk8": slabs8(inputs["Wk"]),
        "Wv8": slabs8(inputs["Wv"]),
        "Wo8": slabs8(inputs["Wo"]),
        "W1_s": slabs16(inputs["W1"], KD),
        "W2_s": slabs16(inputs["W2"], KF),
        "bq_t": cols(np.asarray(inputs["bq"]) * scale, KD),
        "bk_t": cols(inputs["bk"], KD),
        "bv16_t": cols(np.asarray(inputs["bv"]) * CTX8, KD),
        "bo_t": cols(inputs["bo"], KD),
        "b1_t": cols(inputs["b1"], KF),
        "b2_t": cols(inputs["b2"], KD),
        "l1s_t": cols(inputs["ln1_s"], KD),
        "l1b_t": cols(inputs["ln1_b"], KD),
        "l2s_t": cols(inputs["ln2_s"], KD),
        "l2b_t": cols(inputs["ln2_b"], KD),
    }
    prep = {k: np.ascontiguousarray(v) for k, v in prep.items()}

    bias = np.asarray(inputs["sp_table"])[np.asarray(inputs["sp_matrix"])]
    eb = np.exp(bias.astype(np.float64)).astype(f16)
    prep["eb2"] = np.ascontiguousarray(np.concatenate([eb, eb], axis=0))
    prep["id16"] = np.eye(128, dtype=f16)

    # x: [B, S, D] -> per-core feature-major [NCORES, KD, 128, TOK]
    xr = x.reshape(NCORES, TOK, KD, 128)
    xw = np.ascontiguousarray(xr.transpose(0, 2, 3, 1).astype(f16))
    xw8 = np.ascontiguousarray(
        np.clip(xr.transpose(0, 2, 3, 1), -240.0, 240.0).astype(f8))
    return prep, xw, xw8


def kernel(**inputs) -> np.ndarray:
    from concourse import bass_utils

    n_chunks = int(inputs.pop("_n_chunks", TOK // TC))
    trace = bool(inputs.pop("_trace", False))
    stage = inputs.pop("_stage", "full")

    key = (n_chunks, stage)
    if key not in _CACHED_NC:
        _CACHED_NC[key] = _build_nc(n_chunks, stage)
    nc = _CACHED_NC[key]

    prep, xw, xw8 = _host_prep(inputs)
    in_maps = [dict(prep, xw=np.ascontiguousarray(xw[c]),
                    xw8=np.ascontiguousarray(xw8[c]))
               for c in range(NCORES)]

    res = bass_utils.run_bass_kernel_spmd(
        nc, in_maps, core_ids=list(range(NCORES)), trace=trace)
    kernel.last_result = res

    out = np.zeros((B, S, D), dtype=np.float32)
    ntok = n_chunks * TC
    for c in range(NCORES):
        yc = res.results[c]["y"][:ntok].astype(np.float32)
        out[c * BS: c * BS + ntok // S] = yc.reshape(ntok // S, S, D)
    return out
